# revision 3
# baseline (speedup 1.0000x reference)
"""Trainium2 Bass kernel: ContTimeLSTMCell scan (B=64, T=256, H=512).

Strategy: data-parallel over batch (8 cores x 8 rows). Everything on-chip in a
transposed layout: gate/state coordinates on SBUF partitions, batch on the free
dim. Per step the recurrent GEMM is 112 matmuls with stationary fp16 W_h tiles
(fast-weight-load) and the 128x8 transposed hidden state as moving operand,
K-accumulated in PSUM. The x-side GEMM for all timesteps is computed once up
front into an SBUF-resident fp16 tensor. All transcendentals use the single
exp+ln activation table set (sigmoid/tanh via exp + reciprocal) so no table
reloads happen inside the scan.
"""

import sys

for _p in ("/opt/trn_rl_repo",):
    if _p not in sys.path:
        sys.path.insert(0, _p)

import numpy as np

import concourse.bass as bass  # noqa: F401  (import registers bass_rust)
import concourse.mybir as mybir
import concourse.tile as tile
from concourse import bacc, bass_utils
from concourse.alu_op_type import AluOpType as Alu

B, T, H = 64, 256, 512
NCORES = 8
BL = B // NCORES          # 8 batch rows per core
KC = H // 128             # 4 contraction chunks for the h-side (512)
MC = (7 * H) // 128       # 28 gate-coordinate chunks (3584)
GDT = mybir.dt.float16    # GEMM operand dtype
F32 = mybir.dt.float32
AFT = mybir.ActivationFunctionType

# Reference gate order after the fused GEMM: [gi, gf, go, gib, gfb, gd, gz].
# We reorder W's columns to [go, gi, gib, gf, gfb, gz, gd] and pre-scale
# gz by 2 (tanh via sigmoid(2x)) and gd by -1 (softplus via exp(+gd)), so a
# single exp(-x) activation covers every gate.
PERM = [2, 0, 3, 1, 4, 6, 5]
CSCALE = [1.0, 1.0, 1.0, 1.0, 1.0, 2.0, -1.0]


def build_program(t_steps=T):
    nc = bacc.Bacc("TRN2", target_bir_lowering=False, debug=False,
                   num_devices=NCORES)
    ncols = t_steps * BL
    NS = max(1, ncols // 512)         # 512-wide column slices for the x GEMM
    nsw = ncols // NS

    wx_d = nc.dram_tensor("wx", [MC, 128, KC, 128], GDT, kind="ExternalInput").ap()
    wh_d = nc.dram_tensor("wh", [128, KC, MC, 128], GDT, kind="ExternalInput").ap()
    xt_d = nc.dram_tensor("xt", [128, KC, ncols], GDT, kind="ExternalInput").ap()
    dtb_d = nc.dram_tensor("dtb", [128, t_steps, KC * BL], GDT, kind="ExternalInput").ap()
    bias_d = nc.dram_tensor("bias", [128, MC], F32, kind="ExternalInput").ap()

    h_out = nc.dram_tensor("h_out", [t_steps, 128, KC * BL], F32, kind="ExternalOutput").ap()
    cc_out = nc.dram_tensor("cc_out", [t_steps, 128, 2 * KC * BL], F32, kind="ExternalOutput").ap()
    s_out = nc.dram_tensor("s_out", [t_steps, 128, KC * BL], F32, kind="ExternalOutput").ap()
    go_out = nc.dram_tensor("go_out", [t_steps, 128, KC * BL], F32, kind="ExternalOutput").ap()

    W = BL  # batch width shorthand

    with tile.TileContext(nc) as tc:
        with tc.tile_pool(name="const", bufs=1) as constp:
            gx = constp.tile([128, MC, ncols], GDT, tag="gx")
            dtb = constp.tile([128, t_steps, KC * W], GDT, tag="dtb")
            wh = constp.tile([128, KC, MC, 128], GDT, tag="wh")
            bias = constp.tile([128, MC], F32, tag="bias")
            nc.sync.dma_start(dtb[:], dtb_d[:])
            nc.sync.dma_start(wh[:], wh_d[:])
            nc.sync.dma_start(bias[:], bias_d[:])

            # ---- Phase 0: bulk x-side GEMM: gx[m, col] = sum_k Wx^T X^T + b
            with tc.tile_pool(name="xtp", bufs=2) as xtp, \
                 tc.tile_pool(name="wxp", bufs=3) as wxp, \
                 tc.tile_pool(name="ps0", bufs=4, space="PSUM") as ps0:
                for ns in range(NS):
                    xtile = xtp.tile([128, KC, nsw], GDT, tag="xt")
                    nc.sync.dma_start(xtile[:], xt_d[:, :, ns * nsw:(ns + 1) * nsw])
                    for m in range(MC):
                        wxm = wxp.tile([128, KC, 128], GDT, tag="wxm")
                        nc.sync.dma_start(wxm[:], wx_d[m])
                        pt = ps0.tile([128, nsw], F32, tag="pt")
                        for k in range(KC):
                            nc.tensor.matmul(pt[:], wxm[:, k, :],
                                             xtile[:, k, :],
                                             start=(k == 0), stop=(k == KC - 1))
                        nc.vector.tensor_scalar(
                            gx[:, m, ns * nsw:(ns + 1) * nsw], pt[:],
                            bias[:, m:m + 1], None, Alu.add)

            # ---- Phase 1: the scan
            with tc.tile_pool(name="state", bufs=2) as stp, \
                 tc.tile_pool(name="work", bufs=2) as wkp, \
                 tc.tile_pool(name="ps1", bufs=2, space="PSUM") as ps1:
                h16 = stp.tile([128, KC * W], GDT, tag="h16")
                cc = stp.tile([128, 2 * KC * W], F32, tag="cc")  # [c_func | c_bar]
                nc.vector.memset(h16[:], 0.0)
                nc.vector.memset(cc[:], 0.0)

                for t in range(t_steps):
                    pg = ps1.tile([128, MC * W], F32, tag="pg")
                    for m in range(MC):
                        for k in range(KC):
                            nc.tensor.matmul(pg[:, m * W:(m + 1) * W],
                                             wh[:, k, m, :],
                                             h16[:, k * W:(k + 1) * W],
                                             start=(k == 0), stop=(k == KC - 1))

                    # G = pg + gx[t]  (gate pre-activations, transposed)
                    G = wkp.tile([128, MC * W], F32, tag="G")
                    nc.vector.scalar_tensor_tensor(
                        G[:], pg[:], 1.0, gx[:, :, t * W:(t + 1) * W],
                        Alu.mult, Alu.add)
                    # E = exp(-G): sig gates exp(-g), gz exp(-2gz), gd exp(+gd)
                    E = wkp.tile([128, MC * W], F32, tag="E")
                    nc.scalar.activation(E[:], G[:], AFT.Exp, scale=-1.0)
                    D = wkp.tile([128, MC * W], F32, tag="D")
                    nc.scalar.activation(D[:], E[:], AFT.Copy, bias=1.0)
                    # R = 1/(1+exp(-.)) for [go,gi,gib,gf,gfb,gz2]
                    R = wkp.tile([128, 24 * W], F32, tag="R")
                    nc.vector.reciprocal(R[:], D[:, :24 * W])
                    # S = softplus(gd) = ln(1+exp(gd))
                    S = wkp.tile([128, KC * W], F32, tag="S")
                    nc.scalar.activation(S[:], D[:, 24 * W:], AFT.Ln)
                    # z = tanh(gz) = 2*sigmoid(2gz) - 1
                    z = wkp.tile([128, KC * W], F32, tag="z")
                    nc.vector.tensor_scalar(z[:], R[:, 20 * W:24 * W],
                                            2.0, 1.0, Alu.mult, Alu.subtract)
                    # [c_t | cbar_t] = [gf|gfb] * [c_func|c_bar] + [gi|gib] * z
                    t2 = wkp.tile([128, 2 * KC * W], F32, tag="t2")
                    nc.vector.tensor_tensor(t2[:, :KC * W], R[:, 4 * W:8 * W],
                                            z[:], Alu.mult)
                    nc.vector.tensor_tensor(t2[:, KC * W:], R[:, 8 * W:12 * W],
                                            z[:], Alu.mult)
                    t1 = wkp.tile([128, 2 * KC * W], F32, tag="t1")
                    nc.vector.tensor_tensor(t1[:], R[:, 12 * W:20 * W], cc[:],
                                            Alu.mult)
                    ct2 = wkp.tile([128, 2 * KC * W], F32, tag="ct2")
                    nc.vector.tensor_tensor(ct2[:], t1[:], t2[:], Alu.add)
                    # decay = exp(-softplus * dt)
                    du = wkp.tile([128, KC * W], F32, tag="du")
                    nc.vector.tensor_tensor(du[:], S[:], dtb[:, t, :], Alu.mult)
                    dec = wkp.tile([128, KC * W], F32, tag="dec")
                    nc.scalar.activation(dec[:], du[:], AFT.Exp, scale=-1.0)
                    # c_func_t = cbar_t + (c_t - cbar_t) * decay
                    d1 = wkp.tile([128, KC * W], F32, tag="d1")
                    nc.vector.tensor_tensor(d1[:], ct2[:, :KC * W],
                                            ct2[:, KC * W:], Alu.subtract)
                    d2 = wkp.tile([128, KC * W], F32, tag="d2")
                    nc.vector.tensor_tensor(d2[:], d1[:], dec[:], Alu.mult)
                    cc_n = stp.tile([128, 2 * KC * W], F32, tag="cc")
                    nc.vector.tensor_tensor(cc_n[:, :KC * W], d2[:],
                                            ct2[:, KC * W:], Alu.add)
                    nc.vector.tensor_copy(cc_n[:, KC * W:], ct2[:, KC * W:])
                    # h = go * tanh(c_func_t)
                    E2 = wkp.tile([128, KC * W], F32, tag="E2")
                    nc.scalar.activation(E2[:], cc_n[:, :KC * W], AFT.Exp,
                                         scale=-2.0)
                    D2 = wkp.tile([128, KC * W], F32, tag="D2")
                    nc.scalar.activation(D2[:], E2[:], AFT.Copy, bias=1.0)
                    R2 = wkp.tile([128, KC * W], F32, tag="R2")
                    nc.vector.reciprocal(R2[:], D2[:])
                    p1 = wkp.tile([128, KC * W], F32, tag="p1")
                    nc.vector.tensor_tensor(p1[:], R[:, :KC * W], R2[:],
                                            Alu.mult)
                    h32 = wkp.tile([128, KC * W], F32, tag="h32")
                    nc.vector.scalar_tensor_tensor(h32[:], p1[:], 2.0,
                                                   R[:, :KC * W],
                                                   Alu.mult, Alu.subtract)
                    h16 = stp.tile([128, KC * W], GDT, tag="h16")
                    nc.scalar.activation(h16[:], h32[:], AFT.Copy)

                    nc.sync.dma_start(h_out[t], h32[:])
                    nc.sync.dma_start(cc_out[t], ct2[:])
                    nc.sync.dma_start(s_out[t], S[:])
                    nc.sync.dma_start(go_out[t], R[:, :KC * W])
                    cc = cc_n
    nc.compile()
    return nc


def make_in_map(seq_type_embed, dtime, W, b, ci, t_steps=T):
    """Build the per-core input map (all host-side transposes)."""
    Wp = np.concatenate([W[:, 512 * p:512 * (p + 1)] * s
                         for p, s in zip(PERM, CSCALE)], axis=1)
    bp = np.concatenate([b[512 * p:512 * (p + 1)] * s
                         for p, s in zip(PERM, CSCALE)])
    Wx, Wh = Wp[:H], Wp[H:]
    # wh[p, k, m, q] = Wh[128k+p, 128m+q]
    wh = np.ascontiguousarray(
        Wh.reshape(KC, 128, MC, 128).transpose(1, 0, 2, 3)).astype(np.float16)
    # wx[m, p, k, q] = Wx[128k+p, 128m+q]
    wx = np.ascontiguousarray(
        Wx.reshape(KC, 128, MC, 128).transpose(2, 1, 0, 3)).astype(np.float16)
    x = seq_type_embed[ci * BL:(ci + 1) * BL, :t_steps]          # [BL, t, H]
    xt = np.ascontiguousarray(
        x.reshape(BL, t_steps, KC, 128).transpose(3, 2, 1, 0)
        .reshape(128, KC, t_steps * BL)).astype(np.float16)
    dts = dtime[ci * BL:(ci + 1) * BL, :t_steps]                 # [BL, t]
    dtb = np.ascontiguousarray(np.broadcast_to(
        dts.T[None, :, None, :], (128, t_steps, KC, BL))
        .reshape(128, t_steps, KC * BL)).astype(np.float16)
    bias = np.ascontiguousarray(bp.reshape(MC, 128).T).astype(np.float32)
    return {"wx": wx, "wh": wh, "xt": xt, "dtb": dtb, "bias": bias}


def assemble_outputs(results, t_steps=T):
    h_ts = np.empty((B, t_steps, H), np.float32)
    decay = np.empty((B, t_steps, 4, H), np.float32)

    def untrans(a):  # [t, 128, KC*BL] -> [BL, t, H]
        return np.ascontiguousarray(
            a.reshape(t_steps, 128, KC, BL).transpose(3, 0, 2, 1)
            .reshape(BL, t_steps, H))

    for ci, r in enumerate(results):
        sl = slice(ci * BL, (ci + 1) * BL)
        h_ts[sl] = untrans(r["h_out"])
        cc = r["cc_out"]
        decay[sl, :, 0] = untrans(cc[:, :, :KC * BL])
        decay[sl, :, 1] = untrans(cc[:, :, KC * BL:])
        decay[sl, :, 2] = untrans(r["s_out"])
        decay[sl, :, 3] = untrans(r["go_out"])
    return h_ts, decay


_prog_cache = {}


def run(seq_type_embed, dtime, W, b, t_steps=T, trace=False):
    seq_type_embed = np.asarray(seq_type_embed, np.float32)
    dtime = np.asarray(dtime, np.float32)
    W = np.asarray(W, np.float32)
    b = np.asarray(b, np.float32)
    if t_steps not in _prog_cache:
        _prog_cache[t_steps] = build_program(t_steps)
    nc = _prog_cache[t_steps]
    in_maps = [make_in_map(seq_type_embed, dtime, W, b, ci, t_steps)
               for ci in range(NCORES)]
    res = bass_utils.run_bass_kernel_spmd(
        nc, in_maps, core_ids=list(range(NCORES)), trace=trace)
    out = assemble_outputs(res.results, t_steps)
    if trace:
        return out, res
    return out


def kernel(seq_type_embed, dtime, W, b):
    return run(seq_type_embed, dtime, W, b, T)


if __name__ == "__main__":
    # quick smoke test at reduced length
    ts = int(sys.argv[1]) if len(sys.argv) > 1 else 8
    rng = np.random.default_rng(0)
    seq = rng.standard_normal((B, T, H), dtype=np.float32)
    dt = rng.random((B, T), dtype=np.float32)
    Wf = (rng.standard_normal((2 * H, 7 * H), dtype=np.float32)
          / np.sqrt(2.0 * H)).astype(np.float32)
    bf = np.zeros(7 * H, np.float32)
    h, dec = run(seq, dt, Wf, bf, t_steps=ts)
    print("kernel output", h.shape, dec.shape, float(np.abs(h).mean()))
